# revision 47
# baseline (speedup 1.0000x reference)
"""Fused attention kernel for Trainium2 (Bass/Tile), SPMD over 8 NeuronCores.

Problem: B=4, D=64, S=4096 fp32 attention
    A = softmax_k(K^T Q / sqrt(D));  R = V A;  out = concat(R, Q) on channel dim.

Sharding: 8 cores = 4 batches x 2 query-halves (Sq=2048 per core).

Structure (per core): j-outer loop over 32 k-tiles so the PE stationary
operand (k-tile for S = K^T Q, v-tile for R = V E) is loaded once per j and
reused across all q-columns.  Stage (j, h) covers k-rows [128j, 128j+128)
x q-cols [1024h, 1024h+1024).

exp is split across engines (ACT is 1 elem/cycle/lane and would otherwise be
the wall):
  - ACT stages: e = exp(0.125*s + ln(alpha)) via the activation LUT.
  - DVE stages: 2-phase exp2 bit trick.  y1 = round(s*A + B1) as int32 is a
    piecewise-linear approx of alpha1*2^(0.125*s*log2e) when bitcast to f32;
    y2 = y1 + 2^22 is the half-exponent-phase-shifted version.  GPSIMD adds
    the two bitcast-f32 values, averaging the ripple to ~+-1%.  The combined
    scale alpha = mean((f32(y1)+f32(y2)) / exp(t)) is matched on the ACT side
    via the free bias (ln alpha), so softmax normalization cancels it.
R consumes e as float32r (fp22-truncated fp32, 1 PE cycle/row at N>=512).
The softmax divide happens on the host: the kernel ships raw numerator+Z
(V gets a ones-row appended, so Z rides along as output row 64).
"""

import sys

sys.path.insert(0, "/opt/trn_rl_repo")

import numpy as np  # noqa: E402

B, D, S = 4, 64, 4096
NCORES = 8
SQ = S * B // NCORES  # 2048 queries per core
QT = 512              # q-tile width (PE moving operand / psum bank)
KT = 128              # k-tile width
NKT = S // KT         # 32 k-tiles
NQT = SQ // QT        # 4 q-tiles per core
NST = NKT * 2         # 64 stages: (j, h) with h = q-half of 1024 cols
VTW = D + 1           # v-tile width (ones row -> Z)

# DVE fast-exp constants.  t = 0.125*s*log2e;  y1 = round(s*A + B1) int32.
LOG2E = 1.4426950408889634
TRICK_A = float(np.float32(0.125 * LOG2E * (1 << 23)))
TRICK_B1 = float(np.float32((127.0 - 0.0295) * (1 << 23)))
# ln of mean((bitcast_f32(y1) + bitcast_f32(y2)) / exp(t)) over uniform t
ACT_BIAS = 0.900805
# stages handled by the DVE+GPSIMD path (rest on ACT); s % 3 == 1 -> 21/64
DVE_STAGE = [s % 3 == 1 and 8 <= s < 56 for s in range(NST)]

_nc_cache = None


def _build():
    global _nc_cache
    if _nc_cache is not None:
        return _nc_cache
    import concourse.tile as tile
    from concourse import bacc, mybir

    nc = bacc.Bacc(None, target_bir_lowering=False)
    f32 = mybir.dt.float32
    f32r = mybir.dt.float32r
    f16 = mybir.dt.float16
    i32 = mybir.dt.int32
    Alu = mybir.AluOpType

    kst = nc.dram_tensor("kst", [2 * D, S], f16, kind="ExternalInput")
    qrep = nc.dram_tensor("qrep", [2 * D, SQ], f16, kind="ExternalInput")
    vtin = nc.dram_tensor("vtin", [KT, NKT * VTW], f32r, kind="ExternalInput")
    out_rz = nc.dram_tensor("out_rz", [VTW, SQ], f32, kind="ExternalOutput")

    with tile.TileContext(nc) as tc:
        with (
            tc.tile_pool(name="singles", bufs=1) as singles,
            tc.tile_pool(name="sb_e", bufs=9) as sb_e,
            tc.tile_pool(name="sb_y", bufs=4) as sb_y,
            tc.tile_pool(name="sb_o", bufs=1) as sb_o,
            tc.tile_pool(name="ps_s", bufs=2, space="PSUM") as ps_s,
            tc.tile_pool(name="ps_r", bufs=1, space="PSUM") as ps_r,
        ):
            k_sb = singles.tile([2 * D, S], f16)
            q_sb = singles.tile([2 * D, SQ], f16)
            vt_sb = singles.tile([KT, NKT * VTW], f32r)
            wu_a = singles.tile([KT, 16], i32)
            wu_b = singles.tile([KT, 16], i32)
            bias_sb = singles.tile([KT, 1], f32)
            nc.vector.memset(bias_sb, ACT_BIAS)

            r_ps = [
                ps_r.tile([VTW, QT], f32, tag=f"r{t}", name=f"r_ps{t}")
                for t in range(NQT)
            ]

            # Input DMAs, first-needed first.  Stage (j=0, h) needs k-tile 0
            # plus the full q-half h, so q ships in h-halves.
            nc.sync.dma_start(out=k_sb[:, :KT], in_=kst[:, :KT])
            nc.sync.dma_start(out=q_sb[:, :QT], in_=qrep[:, :QT])
            nc.sync.dma_start(out=q_sb[:, QT : 2 * QT], in_=qrep[:, QT : 2 * QT])
            nc.sync.dma_start(out=q_sb[:, 2 * QT :], in_=qrep[:, 2 * QT :])
            nc.scalar.dma_start(out=k_sb[:, KT : 8 * KT], in_=kst[:, KT : 8 * KT])
            nc.scalar.dma_start(out=vt_sb[:, : 4 * VTW], in_=vtin[:, : 4 * VTW])
            for lo, hi in ((8 * KT, 16 * KT), (16 * KT, 24 * KT),
                           (24 * KT, 32 * KT)):
                nc.scalar.dma_start(out=k_sb[:, lo:hi], in_=kst[:, lo:hi])
            nc.scalar.dma_start(out=vt_sb[:, 4 * VTW :], in_=vtin[:, 4 * VTW :])

            # PE warmup: ~3.5us of dummy matmuls on an uninitialized scratch
            # tile during the DMA head keeps the HAM activity window busy so
            # the real matmuls start at 2.4 GHz (results land in r_ps[0] and
            # are discarded by its start=True accumulation reset).
            wu16 = singles.tile([KT, 256], f16)
            nc.vector.memset(wu16, 1.0)
            for _ in range(27):
                nc.tensor.matmul(
                    r_ps[0][:16, :256], wu16[:, :16], wu16[:, :],
                    start=True, stop=True,
                )

            # GPSIMD warmup: force the tensor_tensor ucode IRAM load to happen
            # during the DMA head, not on the first DVE-stage add.
            nc.gpsimd.memset(wu_a, 1)
            nc.gpsimd.memset(wu_b, 2)
            nc.gpsimd.tensor_add(wu_a, wu_a, wu_b)

            from concourse.tile_rust import add_dep_helper

            vt = vt_sb.rearrange("p (j d) -> p j d", j=NKT)

            stage_e = [None] * NST
            stage_s_last = [None] * NST

            def emit_s(s):
                j, h = s // 2, s % 2
                s_ps = ps_s.tile([KT, 2 * QT], f32, tag="s_ps")
                for i in range(2):
                    tq = 2 * h + i
                    mm = nc.tensor.matmul(
                        s_ps[:, i * QT : (i + 1) * QT],
                        k_sb[:, j * KT : (j + 1) * KT],
                        q_sb[:, tq * QT : (tq + 1) * QT],
                        start=True,
                        stop=True,
                    )
                stage_s_last[s] = mm
                e_sb = sb_e.tile([KT, 2 * QT], f32r, tag="e_sb")
                if DVE_STAGE[s]:
                    y1 = sb_y.tile([KT, 2 * QT], i32, tag="y1")
                    nc.vector.tensor_scalar(
                        out=y1, in0=s_ps,
                        scalar1=TRICK_A, scalar2=TRICK_B1,
                        op0=Alu.mult, op1=Alu.add,
                    )
                    y2 = sb_y.tile([KT, 2 * QT], i32, tag="y2")
                    nc.vector.tensor_scalar_add(y2, y1, 1 << 22)
                    nc.gpsimd.tensor_add(e_sb, y1.bitcast(f32), y2.bitcast(f32))
                else:
                    nc.scalar.activation(
                        out=e_sb,
                        in_=s_ps,
                        func=mybir.ActivationFunctionType.Exp,
                        scale=0.125,
                        bias=bias_sb,
                    )
                stage_e[s] = e_sb

            r_touch = [0] * NQT

            def emit_r(s, barrier=None):
                j, h = s // 2, s % 2
                e32r = stage_e[s]
                stage_e[s] = None
                for i in range(2):
                    tq = 2 * h + i
                    # start/stop must follow emission order per psum tile,
                    # which is no longer j-ascending after the reordering
                    mm = nc.tensor.matmul(
                        r_ps[tq],
                        vt[:, j, :],
                        e32r[:, i * QT : (i + 1) * QT],
                        start=(r_touch[tq] == 0),
                        stop=(r_touch[tq] == NKT - 1),
                    )
                    r_touch[tq] += 1
                    if barrier is not None:
                        # Pin R(s) behind S(s+LAG) in the PE queue so the
                        # tile scheduler cannot collapse the pipeline depth
                        # below the DVE->GPSIMD exp-chain latency.
                        add_dep_helper(
                            mm.ins, barrier.ins, sync=True,
                            reason="hold R-matmul back for exp-chain latency",
                        )

            # Pair-grouped emission: all 4 S-matmuls of k-tile j are adjacent
            # in the PE queue (one stationary load), then all 4 R-matmuls of
            # the stage RLAG positions back (one vt load).  The tail runs the
            # last 4 k-tiles h=0-first so r_ps[0..1] close ~4 stages before
            # the end and their output copy+DMA overlap the final h=1 work.
            TJ = NKT - 4
            seq = [(j, h) for j in range(TJ) for h in (0, 1)]
            seq += [(j, 0) for j in range(TJ, NKT)]
            seq += [(j, 1) for j in range(TJ, NKT)]
            RLAG = 6
            for i, (j, h) in enumerate(seq):
                emit_s(2 * j + h)
                if i >= RLAG:
                    pj, ph = seq[i - RLAG]
                    emit_r(2 * pj + ph, barrier=stage_s_last[2 * j + h])
            for i in range(len(seq) - RLAG, len(seq)):
                pj, ph = seq[i]
                emit_r(2 * pj + ph)

            rz_sb = sb_o.tile([VTW, SQ], f32, tag="rz")
            for tq in range(NQT):
                if tq % 2 == 0:
                    nc.scalar.copy(
                        out=rz_sb[:, tq * QT : (tq + 1) * QT], in_=r_ps[tq]
                    )
                else:
                    nc.vector.tensor_copy(
                        out=rz_sb[:, tq * QT : (tq + 1) * QT], in_=r_ps[tq]
                    )
                eng = (nc.sync, nc.gpsimd, nc.sync, nc.gpsimd)[tq]
                eng.dma_start(
                    out=out_rz[:, tq * QT : (tq + 1) * QT],
                    in_=rz_sb[:, tq * QT : (tq + 1) * QT],
                )

    nc.compile()
    _nc_cache = nc
    return nc


def _in_maps(K, V, Q):
    K = np.asarray(K, dtype=np.float32)
    V = np.asarray(V, dtype=np.float32)
    Q = np.asarray(Q, dtype=np.float32)
    maps = []
    for c in range(NCORES):
        b, h = c // 2, c % 2
        khi = K[b].astype(np.float16)
        klo = (K[b] - khi.astype(np.float32)).astype(np.float16)
        kst = np.concatenate([khi, klo], axis=0)  # [128, S]
        qhi = Q[b, :, h * SQ : (h + 1) * SQ].astype(np.float16)
        qrep = np.concatenate([qhi, qhi], axis=0)  # [128, SQ]
        # v-tiles: vt[p, j, d] = V[b, d, KT*j + p]; vt[p, j, D] = 1.0
        vt = np.empty((KT, NKT, VTW), dtype=np.float32)
        vt[:, :, :D] = V[b].T.reshape(NKT, KT, D).transpose(1, 0, 2)
        vt[:, :, D] = 1.0
        maps.append(
            {
                "kst": np.ascontiguousarray(kst),
                "qrep": np.ascontiguousarray(qrep),
                "vtin": np.ascontiguousarray(vt.reshape(KT, NKT * VTW)),
            }
        )
    return maps


def _run(K, V, Q, trace=False):
    from concourse.bass_utils import run_bass_kernel_spmd

    nc = _build()
    res = run_bass_kernel_spmd(
        nc, _in_maps(K, V, Q), list(range(NCORES)), trace=trace
    )
    Q = np.asarray(Q, dtype=np.float32)
    out = np.empty((B, 2 * D, S), dtype=np.float32)
    out[:, D : 2 * D, :] = Q
    for c in range(NCORES):
        b, h = c // 2, c % 2
        rz = res.results[c]["out_rz"].astype(np.float64)
        out[b, 0:D, h * SQ : (h + 1) * SQ] = (
            rz[0:D] / rz[D : D + 1]
        ).astype(np.float32)
    return out, res


def kernel(K, V, Q):
    out, _ = _run(K, V, Q, trace=False)
    return out


# revision 48
# speedup vs baseline: 1.1802x; 1.1802x over previous
"""Fused attention kernel for Trainium2 (Bass/Tile), SPMD over 8 NeuronCores.

Problem: B=4, D=64, S=4096 fp32 attention
    A = softmax_k(K^T Q / sqrt(D));  R = V A;  out = concat(R, Q) on channel dim.

Sharding: 8 cores = 4 batches x 2 query-halves (Sq=2048 per core).

Structure (per core): j-outer loop over 32 k-tiles so the PE stationary
operand (k-tile for S = K^T Q, v-tile for R = V E) is loaded once per j and
reused across all q-columns.  Stage (j, h) covers k-rows [128j, 128j+128)
x q-cols [1024h, 1024h+1024).

exp is split across engines (ACT is 1 elem/cycle/lane and would otherwise be
the wall):
  - ACT stages: e = exp(0.125*s + ln(alpha)) via the activation LUT.
  - DVE stages: 2-phase exp2 bit trick.  y1 = round(s*A + B1) as int32 is a
    piecewise-linear approx of alpha1*2^(0.125*s*log2e) when bitcast to f32;
    y2 = y1 + 2^22 is the half-exponent-phase-shifted version.  GPSIMD adds
    the two bitcast-f32 values, averaging the ripple to ~+-1%.  The combined
    scale alpha = mean((f32(y1)+f32(y2)) / exp(t)) is matched on the ACT side
    via the free bias (ln alpha), so softmax normalization cancels it.
R consumes e as float32r (fp22-truncated fp32, 1 PE cycle/row at N>=512).
The softmax divide happens on the host: the kernel ships raw numerator+Z
(V gets a ones-row appended, so Z rides along as output row 64).
"""

import sys

sys.path.insert(0, "/opt/trn_rl_repo")

import numpy as np  # noqa: E402

B, D, S = 4, 64, 4096
NCORES = 8
SQ = S * B // NCORES  # 2048 queries per core
QT = 512              # q-tile width (PE moving operand / psum bank)
KT = 128              # k-tile width
NKT = S // KT         # 32 k-tiles
NQT = SQ // QT        # 4 q-tiles per core
NST = NKT * 2         # 64 stages: (j, h) with h = q-half of 1024 cols
VTW = D + 1           # v-tile width (ones row -> Z)

# DVE fast-exp constants.  t = 0.125*s*log2e;  y1 = round(s*A + B1) int32.
LOG2E = 1.4426950408889634
TRICK_A = float(np.float32(0.125 * LOG2E * (1 << 23)))
TRICK_B1 = float(np.float32((127.0 - 0.0295) * (1 << 23)))
# ln of mean((bitcast_f32(y1) + bitcast_f32(y2)) / exp(t)) over uniform t
ACT_BIAS = 0.900805
# stages handled by the DVE+GPSIMD path (rest on ACT); s % 3 == 1 -> 21/64
DVE_STAGE = [s % 3 == 1 and s < 52 for s in range(NST)]

_nc_cache = None


def _build():
    global _nc_cache
    if _nc_cache is not None:
        return _nc_cache
    import concourse.tile as tile
    from concourse import bacc, mybir

    nc = bacc.Bacc(None, target_bir_lowering=False)
    f32 = mybir.dt.float32
    f32r = mybir.dt.float32r
    f16 = mybir.dt.float16
    i32 = mybir.dt.int32
    Alu = mybir.AluOpType

    kst = nc.dram_tensor("kst", [2 * D, S], f16, kind="ExternalInput")
    qrep = nc.dram_tensor("qrep", [2 * D, SQ], f16, kind="ExternalInput")
    vtin = nc.dram_tensor("vtin", [KT, NKT * VTW], f32r, kind="ExternalInput")
    out_rz = nc.dram_tensor("out_rz", [VTW, SQ], f32, kind="ExternalOutput")

    with tile.TileContext(nc) as tc:
        with (
            tc.tile_pool(name="singles", bufs=1) as singles,
            tc.tile_pool(name="sb_e", bufs=9) as sb_e,
            tc.tile_pool(name="sb_y", bufs=4) as sb_y,
            tc.tile_pool(name="sb_o", bufs=1) as sb_o,
            tc.tile_pool(name="ps_s", bufs=2, space="PSUM") as ps_s,
            tc.tile_pool(name="ps_r", bufs=1, space="PSUM") as ps_r,
        ):
            k_sb = singles.tile([2 * D, S], f16)
            q_sb = singles.tile([2 * D, SQ], f16)
            vt_sb = singles.tile([KT, NKT * VTW], f32r)
            wu_a = singles.tile([KT, 16], i32)
            wu_b = singles.tile([KT, 16], i32)
            bias_sb = singles.tile([KT, 1], f32)
            nc.vector.memset(bias_sb, ACT_BIAS)

            r_ps = [
                ps_r.tile([VTW, QT], f32, tag=f"r{t}", name=f"r_ps{t}")
                for t in range(NQT)
            ]

            # Input DMAs, first-needed first.  Stage (j=0, h) needs k-tile 0
            # plus the full q-half h, so q ships in h-halves.
            nc.sync.dma_start(out=k_sb[:, :KT], in_=kst[:, :KT])
            nc.sync.dma_start(out=q_sb[:, :QT], in_=qrep[:, :QT])
            nc.sync.dma_start(out=q_sb[:, QT : 2 * QT], in_=qrep[:, QT : 2 * QT])
            nc.sync.dma_start(out=q_sb[:, 2 * QT :], in_=qrep[:, 2 * QT :])
            nc.scalar.dma_start(out=k_sb[:, KT : 8 * KT], in_=kst[:, KT : 8 * KT])
            nc.scalar.dma_start(out=vt_sb[:, : 4 * VTW], in_=vtin[:, : 4 * VTW])
            for lo, hi in ((8 * KT, 16 * KT), (16 * KT, 24 * KT),
                           (24 * KT, 32 * KT)):
                nc.scalar.dma_start(out=k_sb[:, lo:hi], in_=kst[:, lo:hi])
            nc.scalar.dma_start(out=vt_sb[:, 4 * VTW :], in_=vtin[:, 4 * VTW :])

            # PE warmup: ~3.5us of dummy matmuls on an uninitialized scratch
            # tile during the DMA head keeps the HAM activity window busy so
            # the real matmuls start at 2.4 GHz (results land in r_ps[0] and
            # are discarded by its start=True accumulation reset).
            wu16 = singles.tile([KT, 256], f16)
            nc.vector.memset(wu16, 1.0)
            for _ in range(27):
                nc.tensor.matmul(
                    r_ps[0][:16, :256], wu16[:, :16], wu16[:, :],
                    start=True, stop=True,
                )

            # GPSIMD warmup: force the tensor_tensor ucode IRAM load to happen
            # during the DMA head, not on the first DVE-stage add.
            nc.gpsimd.memset(wu_a, 1)
            nc.gpsimd.memset(wu_b, 2)
            nc.gpsimd.tensor_add(wu_a, wu_a, wu_b)

            from concourse.tile_rust import add_dep_helper

            vt = vt_sb.rearrange("p (j d) -> p j d", j=NKT)

            stage_e = [None] * NST
            stage_s_last = [None] * NST

            def emit_s(s):
                j, h = s // 2, s % 2
                s_ps = ps_s.tile([KT, 2 * QT], f32, tag="s_ps")
                for i in range(2):
                    tq = 2 * h + i
                    mm = nc.tensor.matmul(
                        s_ps[:, i * QT : (i + 1) * QT],
                        k_sb[:, j * KT : (j + 1) * KT],
                        q_sb[:, tq * QT : (tq + 1) * QT],
                        start=True,
                        stop=True,
                    )
                stage_s_last[s] = mm
                e_sb = sb_e.tile([KT, 2 * QT], f32r, tag="e_sb")
                if DVE_STAGE[s]:
                    y1 = sb_y.tile([KT, 2 * QT], i32, tag="y1")
                    nc.vector.tensor_scalar(
                        out=y1, in0=s_ps,
                        scalar1=TRICK_A, scalar2=TRICK_B1,
                        op0=Alu.mult, op1=Alu.add,
                    )
                    y2 = sb_y.tile([KT, 2 * QT], i32, tag="y2")
                    nc.vector.tensor_scalar_add(y2, y1, 1 << 22)
                    nc.gpsimd.tensor_add(e_sb, y1.bitcast(f32), y2.bitcast(f32))
                else:
                    nc.scalar.activation(
                        out=e_sb,
                        in_=s_ps,
                        func=mybir.ActivationFunctionType.Exp,
                        scale=0.125,
                        bias=bias_sb,
                    )
                stage_e[s] = e_sb

            r_touch = [0] * NQT

            def emit_r(s, barrier=None):
                j, h = s // 2, s % 2
                e32r = stage_e[s]
                stage_e[s] = None
                for i in range(2):
                    tq = 2 * h + i
                    # start/stop must follow emission order per psum tile,
                    # which is no longer j-ascending after the reordering
                    mm = nc.tensor.matmul(
                        r_ps[tq],
                        vt[:, j, :],
                        e32r[:, i * QT : (i + 1) * QT],
                        start=(r_touch[tq] == 0),
                        stop=(r_touch[tq] == NKT - 1),
                    )
                    r_touch[tq] += 1
                    if barrier is not None:
                        # Pin R(s) behind S(s+LAG) in the PE queue so the
                        # tile scheduler cannot collapse the pipeline depth
                        # below the DVE->GPSIMD exp-chain latency.
                        add_dep_helper(
                            mm.ins, barrier.ins, sync=True,
                            reason="hold R-matmul back for exp-chain latency",
                        )

            # Pair-grouped emission: all 4 S-matmuls of k-tile j are adjacent
            # in the PE queue (one stationary load), then all 4 R-matmuls of
            # the stage RLAG positions back (one vt load).  The tail runs the
            # last 4 k-tiles h=0-first so r_ps[0..1] close ~4 stages before
            # the end and their output copy+DMA overlap the final h=1 work.
            TJ = NKT - 4
            seq = [(j, h) for j in range(TJ) for h in (0, 1)]
            seq += [(j, 0) for j in range(TJ, NKT)]
            seq += [(j, 1) for j in range(TJ, NKT)]
            RLAG = 6
            for i, (j, h) in enumerate(seq):
                emit_s(2 * j + h)
                if i >= RLAG:
                    pj, ph = seq[i - RLAG]
                    emit_r(2 * pj + ph, barrier=stage_s_last[2 * j + h])
            for i in range(len(seq) - RLAG, len(seq)):
                pj, ph = seq[i]
                emit_r(2 * pj + ph)

            rz_sb = sb_o.tile([VTW, SQ], f32, tag="rz")
            for tq in range(NQT):
                if tq % 2 == 0:
                    nc.scalar.copy(
                        out=rz_sb[:, tq * QT : (tq + 1) * QT], in_=r_ps[tq]
                    )
                else:
                    nc.vector.tensor_copy(
                        out=rz_sb[:, tq * QT : (tq + 1) * QT], in_=r_ps[tq]
                    )
                eng = (nc.sync, nc.gpsimd, nc.sync, nc.gpsimd)[tq]
                eng.dma_start(
                    out=out_rz[:, tq * QT : (tq + 1) * QT],
                    in_=rz_sb[:, tq * QT : (tq + 1) * QT],
                )

    nc.compile()
    _nc_cache = nc
    return nc


def _in_maps(K, V, Q):
    K = np.asarray(K, dtype=np.float32)
    V = np.asarray(V, dtype=np.float32)
    Q = np.asarray(Q, dtype=np.float32)
    maps = []
    for c in range(NCORES):
        b, h = c // 2, c % 2
        khi = K[b].astype(np.float16)
        klo = (K[b] - khi.astype(np.float32)).astype(np.float16)
        kst = np.concatenate([khi, klo], axis=0)  # [128, S]
        qhi = Q[b, :, h * SQ : (h + 1) * SQ].astype(np.float16)
        qrep = np.concatenate([qhi, qhi], axis=0)  # [128, SQ]
        # v-tiles: vt[p, j, d] = V[b, d, KT*j + p]; vt[p, j, D] = 1.0
        vt = np.empty((KT, NKT, VTW), dtype=np.float32)
        vt[:, :, :D] = V[b].T.reshape(NKT, KT, D).transpose(1, 0, 2)
        vt[:, :, D] = 1.0
        maps.append(
            {
                "kst": np.ascontiguousarray(kst),
                "qrep": np.ascontiguousarray(qrep),
                "vtin": np.ascontiguousarray(vt.reshape(KT, NKT * VTW)),
            }
        )
    return maps


def _run(K, V, Q, trace=False):
    from concourse.bass_utils import run_bass_kernel_spmd

    nc = _build()
    res = run_bass_kernel_spmd(
        nc, _in_maps(K, V, Q), list(range(NCORES)), trace=trace
    )
    Q = np.asarray(Q, dtype=np.float32)
    out = np.empty((B, 2 * D, S), dtype=np.float32)
    out[:, D : 2 * D, :] = Q
    for c in range(NCORES):
        b, h = c // 2, c % 2
        rz = res.results[c]["out_rz"].astype(np.float64)
        out[b, 0:D, h * SQ : (h + 1) * SQ] = (
            rz[0:D] / rz[D : D + 1]
        ).astype(np.float32)
    return out, res


def kernel(K, V, Q):
    out, _ = _run(K, V, Q, trace=False)
    return out
